# revision 43
# baseline (speedup 1.0000x reference)
"""SO3Conv Trainium2 Bass kernel.

Math (per reference):
  psi[f,g,i] = sum_n D[n,i] w[f,g,n] / sqrt(64)
  per l (d=2l+1, blk=d*d at offset off):
    y[b,g,off+v*d+m] = 1/sqrt(64*d) * sum_{f,u} x[b,f,off+u*d+m] * psi[f,g,off+u*d+v]

Strategy: data-parallel over batch (8 cores x 128 batch).
Per core:
  A) Host pre-transposes x into per-l lhsT layouts (no on-device XBAR):
     xf_l [(uin f), (m ku b)] bf16 for full 128-row K-slabs (u pairs) and
     xh_l [f, (m b)] for the odd last-u 64-row slabs.  Plain linear DMAs
     at full bandwidth, no padding.
  B) Host provides wT [n, (f g)] and d_pre [n, i'] (pre-scaled per l by
     1/(64 sqrt d), columns reordered l-descending) directly in bf16.
  C) psi computed on PE in psiT layout [i'-chunk-part, (f g)-free], 4
     chunks x 8 matmuls (i' cuts at u-row boundaries), parked to DRAM
     scratch, read back into per-l rhs tiles [(u-parity,f)-part,
     (ku,v,g)-free] via strided gathers.
  D) main matmuls run mg(m-group)-outer / ku-inner over 8 PSUM banks;
     PSUM [b,(v g)] fp32 copied (cast bf16) contiguously into per-l y
     tiles in [b, (m, v, g)] order; stored bf16 to DRAM per m-group;
     host converts to fp32 and reassembles.
  Queues: sync carries the latency-critical input chain in FIFO priority
  order (d_pre, wT, x-l6, psi parks + readbacks, remaining x); scalar
  carries y stores; PSUM->SBUF copies rotate scalar/vector/gpsimd.
"""

import sys

sys.path.insert(0, "/opt/trn_rl_repo")

import numpy as np

LMAX = 6
F = 64
NROT = 64
IRREP = 455
B = 1024
NCORES = 8
BS = B // NCORES  # 128

DS = [2 * l + 1 for l in range(LMAX + 1)]
OFFS = []
_o = 0
for _d in DS:
    OFFS.append(_o)
    _o += _d * _d
assert _o == IRREP

LORDER = list(range(LMAX, -1, -1))  # process l descending

# reordered i-space for d_pre columns / psiT rows: small l first (their
# main-loop processing starts first), then l6/l5 blocks, l4 u0-7 last;
# l4-u8 rides in chunk 0.
UORDER = (
    [(6, u) for u in range(13)]
    + [(5, u) for u in range(11)]
    + [(4, u) for u in range(9)]
    + [(3, u) for u in range(7)]
    + [(2, u) for u in range(5)]
    + [(1, u) for u in range(3)]
    + [(0, 0)]
)
RPOS = {}
_o = 0
for _lu in UORDER:
    RPOS[_lu] = _o
    _o += DS[_lu[0]]
assert _o == IRREP

# psi matmul chunks in reordered i-space; cuts at u-row boundaries
CHUNKS = [(0, 117), (117, 235), (235, 362), (362, 455)]
PARKROWS = [117, 118, 127, 93]

# main-loop processing order and m-group sizes: small l first (their
# park/readback chains are ready first and their compute covers the
# streaming of l6's chain), l0 last for a tiny tail
LPROC = [6, 5, 4, 3, 2, 1, 0]
MGS = {6: 4, 5: 4, 4: 4, 3: 8, 2: 8, 1: 8, 0: 8}

# y DRAM region offsets (l descending), cols per l = d*d*64, order (m,v,g)
YLEN = {l: DS[l] * DS[l] * 64 for l in LORDER}
YOFF = {}
_o = 0
for l in LORDER:
    YOFF[l] = _o
    _o += YLEN[l]
YTOT = _o  # 29120

# x chunk counts
NCHF = {l: DS[l] * (DS[l] // 2) for l in LORDER}  # full 128-row chunks
# sm (l3..l0) combined chunk bases
CBF = {}
_c = 0
for l in (3, 2, 1, 0):
    CBF[l] = _c
    _c += NCHF[l]
NCHF_SM = _c  # 34
HBF = {}
_c = 0
for l in (3, 2, 1, 0):
    HBF[l] = _c
    _c += DS[l]
NH_SM = _c  # 16

_CACHE = {}


def _build():
    import concourse.bacc as bacc
    import concourse.bass as bass
    import concourse.mybir as mybir
    from concourse import tile

    dt = mybir.dt
    BF = dt.bfloat16
    F32 = dt.float32

    nc = bacc.Bacc("TRN2", target_bir_lowering=False, debug=False, num_devices=NCORES)

    dpre_d = nc.dram_tensor("dpre", [NROT, IRREP], BF, kind="ExternalInput")
    wt_d = nc.dram_tensor("wt", [NROT, F * F], BF, kind="ExternalInput")
    xf_d = {
        l: nc.dram_tensor(f"xf{l}", [128, NCHF[l] * BS], BF, kind="ExternalInput")
        for l in (6, 5, 4)
    }
    xh_d = {
        l: nc.dram_tensor(f"xh{l}", [64, DS[l] * BS], BF, kind="ExternalInput")
        for l in (6, 5, 4)
    }
    xf_d["sm"] = nc.dram_tensor("xfsm", [128, NCHF_SM * BS], BF, kind="ExternalInput")
    xh_d["sm"] = nc.dram_tensor("xhsm", [64, NH_SM * BS], BF, kind="ExternalInput")
    y_d = nc.dram_tensor("y", [BS, YTOT], BF, kind="ExternalOutput")
    park_t = [
        nc.dram_tensor(f"psiS{ci}", [PARKROWS[ci], F * F], BF) for ci in range(4)
    ]

    eng_flip = [0]

    with tile.TileContext(nc) as tc:
        with (
            tc.tile_pool(name="const", bufs=1) as cp,
            tc.tile_pool(name="xt", bufs=1) as xp,
            tc.tile_pool(name="rhs", bufs=1) as rp,
            tc.tile_pool(name="yb", bufs=1) as yp,
            tc.tile_pool(name="psit", bufs=1) as psp,
        ):
            # ---- persistent tiles ----
            wT = cp.tile([NROT, F * F], BF, name="wT", tag="wT")
            d_pre = cp.tile([NROT, IRREP], BF, name="dpre", tag="dpre")
            xtF = {}
            xtH = {}
            rhs = {}
            yb = {}
            for l in (6, 5, 4):
                xtF[l] = xp.tile([128, NCHF[l], BS], BF, name=f"xtF{l}", tag=f"xtF{l}")
                xtH[l] = xp.tile([64, DS[l], BS], BF, name=f"xtH{l}", tag=f"xtH{l}")
                yb[l] = yp.tile([BS, YLEN[l]], BF, name=f"yb{l}", tag=f"yb{l}")
            xtF["sm"] = xp.tile([128, NCHF_SM, BS], BF, name="xtFsm", tag="xtFsm")
            xtH["sm"] = xp.tile([64, NH_SM, BS], BF, name="xtHsm", tag="xtHsm")
            YSM = sum(YLEN[l] for l in (3, 2, 1, 0))  # 5376
            yb["sm"] = yp.tile([BS, YSM], BF, name="ybsm", tag="ybsm")
            YB = {l: YOFF[l] - YOFF[3] for l in (3, 2, 1, 0)}
            # ku-slab stride padded by one g-block so (ku,v) dims cannot
            # merge in AP canonicalization (DMA balancer needs them split)
            KS = {l: (DS[l] + 1) * 64 for l in LORDER}
            for l in LORDER:
                d = DS[l]
                nku = (d + 1) // 2
                rhs[l] = rp.tile(
                    [128, nku * KS[l]], BF, name=f"rhs{l}", tag=f"rhs{l}"
                )
            psiT = [
                psp.tile([128, F * F], BF, name=f"psiT{ci}", tag=f"psiT{ci}")
                for ci in range(4)
            ]

            # ---- emission helpers ----
            def xfload(l, c0, c1):
                nc.sync.dma_start(
                    xtF[l][:, c0:c1, :],
                    xf_d[l][:, c0 * BS : c1 * BS].rearrange(
                        "p (c b) -> p c b", b=BS
                    ),
                )

            def xhload(l):
                n = DS[l] if l != "sm" else NH_SM
                nc.sync.dma_start(
                    xtH[l][:, :, :],
                    xh_d[l].rearrange("f (m b) -> f m b", b=BS),
                )

            def park(ci, a, b, q=None):
                (q or nc.sync).dma_start(park_t[ci][a:b, :], psiT[ci][a:b, :])

            def _chunk_of(gi):
                for ci, (r0, r1) in enumerate(CHUNKS):
                    if r0 <= gi < r1:
                        return ci, gi - r0
                raise AssertionError(gi)

            def rb_single(l, u, q=None):
                d = DS[l]
                ku, uin = divmod(u, 2)
                ci, a = _chunk_of(RPOS[(l, u)])
                dst = rhs[l][
                    uin * 64 : (uin + 1) * 64, ku * KS[l] : ku * KS[l] + d * 64
                ].rearrange("f (v g) -> f v g", g=64)
                sv = park_t[ci].rearrange("i (f g) -> f i g", g=64)
                (q or nc.sync).dma_start(dst, sv[:, a : a + d, :])

            def rb_us(l, us, q=None):
                for u in us:
                    rb_single(l, u, q)

            def copy(dst, src):
                if eng_flip[0] % 2 == 0:
                    nc.scalar.copy(dst, src)
                else:
                    nc.vector.tensor_copy(dst, src)
                eng_flip[0] += 1

            # ---- sync queue head: constants ----
            nc.sync.dma_start(d_pre[:, :], dpre_d[:, :])
            nc.sync.dma_start(wT[:, :512], wt_d[:, :512])
            nc.sync.dma_start(wT[:, 512:1024], wt_d[:, 512:1024])
            nc.sync.dma_start(wT[:, 1024:2048], wt_d[:, 1024:2048])
            nc.sync.dma_start(wT[:, 2048:], wt_d[:, 2048:])

            # ---- PE primer: dummy matmuls ramp the PE p-state and keep it
            # busy until the first psi matmul's inputs land (~3.2us) ----
            dummy = cp.tile([128, 384], BF, name="dummy", tag="dummy")
            nc.vector.memset(dummy[:, :], 0.0)
            with tc.tile_pool(name="pw", bufs=1, space=bass.MemorySpace.PSUM) as pw:
                warm = pw.tile([128, 512], F32, tag="warm", name="warm")
                for _ in range(9):
                    nc.tensor.matmul(
                        warm[:, :256],
                        dummy[:, :128],
                        dummy[:, 128:],
                        start=True,
                        stop=True,
                    )

            # ---- psi matmuls (PE) + copies ----
            with tc.tile_pool(name="pa", bufs=2, space=bass.MemorySpace.PSUM) as pa:
                for ci, (r0, r1) in enumerate(CHUNKS):
                    rows = r1 - r0
                    for p in range(2):
                        pps = pa.tile(
                            [128, 2048], F32, tag="pps", name=f"pps{ci}_{p}"
                        )
                        for h in range(4):
                            s = 4 * p + h
                            nc.tensor.matmul(
                                pps[:rows, h * 512 : (h + 1) * 512],
                                d_pre[:, r0:r1],
                                wT[:, s * 512 : (s + 1) * 512],
                                start=True,
                                stop=True,
                            )
                        copy(psiT[ci][:rows, p * 2048 : (p + 1) * 2048], pps[:rows, :])

            # ---- sync queue: x + park/readback chain (priority order) ----
            xfload(6, 0, 24)       # l6 mg0 (m0-3, ku0-5)
            xhload(6)
            park(0, 0, 26)         # l6 u0,u1
            rb_us(6, (0, 1))
            park(0, 26, 117)       # l6 u2..u8
            rb_us(6, (2, 3))
            xfload(6, 24, 48)      # mg1
            rb_us(6, (4, 5, 6, 7))
            park(1, 0, 52)         # l6 u9..u12
            rb_us(6, (8, 9, 10, 11, 12))
            xfload(6, 48, 78)      # mg2+mg3
            park(1, 52, 118)       # l5 u0..u5
            rb_us(5, range(0, 6))
            park(2, 0, 55)         # l5 u6..u10
            rb_us(5, range(6, 11))
            xfload(5, 0, 28)       # l5 mg0 (+)
            xhload(5)
            park(2, 55, 127)       # l4 u0..u7
            rb_us(4, range(0, 8))
            xfload(5, 28, 55)
            park(3, 0, 93)         # l4 u8 + l3..l0
            rb_single(4, 8)
            xfload(4, 0, 36)
            xhload(4)
            xfload("sm", 0, NCHF_SM)
            xhload("sm")
            # small-l readbacks on the SWDGE (Pool) queue - no HWDGE slot
            for _l in (3, 2, 1, 0):
                rb_us(_l, range(DS[_l]), nc.gpsimd)

            # ---- main loop ----
            with tc.tile_pool(name="py", bufs=1, space=bass.MemorySpace.PSUM) as py:
                for l in LPROC:
                    d = DS[l]
                    nku = (d + 1) // 2
                    if d * 64 <= 512:
                        vsplits = [(0, d)]
                    else:
                        vsplits = [(0, 8), (8, d - 8)]
                    mg_size = MGS[l]
                    xftile = xtF[l] if l >= 4 else xtF["sm"]
                    xhtile = xtH[l] if l >= 4 else xtH["sm"]
                    cf = 0 if l >= 4 else CBF[l]
                    ch = 0 if l >= 4 else HBF[l]
                    ytile = yb[l] if l >= 4 else yb["sm"]
                    ybase = 0 if l >= 4 else YB[l]
                    for mg0 in range(0, d, mg_size):
                        ms = list(range(mg0, min(d, mg0 + mg_size)))
                        pyt = {}
                        for m in ms:
                            for vi, (v0, nv) in enumerate(vsplits):
                                slot = (m - mg0) * len(vsplits) + vi
                                pyt[(m, v0)] = py.tile(
                                    [BS, 512], F32, tag=f"py{slot}",
                                    name=f"py{l}_{m}_{v0}",
                                )
                        for ku in range(nku):
                            for m in ms:
                                if ku < d // 2:
                                    lhsT = xftile[:, cf + m * (d // 2) + ku, :]
                                    kk = 128
                                else:
                                    lhsT = xhtile[:, ch + m, :]
                                    kk = 64
                                for (v0, nv) in vsplits:
                                    nc.tensor.matmul(
                                        pyt[(m, v0)][:, : nv * 64],
                                        lhsT,
                                        rhs[l][
                                            :kk,
                                            ku * KS[l]
                                            + v0 * 64 : ku * KS[l]
                                            + (v0 + nv) * 64,
                                        ],
                                        start=(ku == 0),
                                        stop=(ku == nku - 1),
                                    )
                        for m in ms:
                            for (v0, nv) in vsplits:
                                copy(
                                    ytile[
                                        :,
                                        ybase
                                        + (m * d + v0) * 64 : ybase
                                        + (m * d + v0 + nv) * 64,
                                    ],
                                    pyt[(m, v0)][:, : nv * 64],
                                )
                        nc.scalar.dma_start(
                            y_d[
                                :,
                                YOFF[l] + ms[0] * d * 64 : YOFF[l]
                                + (ms[-1] + 1) * d * 64,
                            ],
                            ytile[
                                :,
                                ybase + ms[0] * d * 64 : ybase
                                + (ms[-1] + 1) * d * 64,
                            ],
                        )

    nc.compile()
    return nc


def _get_nc():
    if "nc" not in _CACHE:
        _CACHE["nc"] = _build()
    return _CACHE["nc"]


def _xfh(xc, l):
    """xc [BS, F, IRREP] fp32 -> (xf [(uin f),(m ku b)], xh [f,(m b)]) fp32."""
    d = DS[l]
    xl = xc[:, :, OFFS[l] : OFFS[l] + d * d].reshape(BS, F, d, d)  # [b,f,u,m]
    xf = None
    if d // 2:
        a = xl[:, :, : 2 * (d // 2), :].reshape(BS, F, d // 2, 2, d)
        # [b, f, ku, uin, m] -> [uin, f, m, ku, b]
        xf = np.ascontiguousarray(a.transpose(3, 1, 4, 2, 0)).reshape(
            128, d * (d // 2) * BS
        )
    xh = np.ascontiguousarray(
        xl[:, :, d - 1, :].transpose(1, 2, 0)
    ).reshape(F, d * BS)
    return xf, xh


def _prep_x(xc, bf16):
    out = {}
    sm_f, sm_h = [], []
    for l in LORDER:
        xf, xh = _xfh(xc, l)
        if l >= 4:
            out[f"xf{l}"] = xf.astype(bf16)
            out[f"xh{l}"] = xh.astype(bf16)
        else:
            if xf is not None:
                sm_f.append(xf)
            sm_h.append(xh)
    out["xfsm"] = np.concatenate(sm_f, axis=1).astype(bf16)
    out["xhsm"] = np.concatenate(sm_h, axis=1).astype(bf16)
    return out


def kernel(x, D, w):
    import ml_dtypes
    from concourse.bass_utils import run_bass_kernel_spmd

    bf16 = ml_dtypes.bfloat16
    nc = _get_nc()

    wt = np.ascontiguousarray(
        np.asarray(w, dtype=np.float32).transpose(2, 0, 1)
    ).reshape(NROT, F * F).astype(bf16)
    Dn = np.asarray(D, dtype=np.float32)
    dp = np.empty((NROT, IRREP), dtype=np.float32)
    for (l, u), r in RPOS.items():
        d = DS[l]
        dp[:, r : r + d] = Dn[
            :, OFFS[l] + u * d : OFFS[l] + (u + 1) * d
        ] / (64.0 * np.sqrt(d))
    dpre = dp.astype(bf16)

    in_maps = []
    for c in range(NCORES):
        m = {"dpre": dpre, "wt": wt}
        m.update(
            _prep_x(np.asarray(x[c * BS : (c + 1) * BS], dtype=np.float32), bf16)
        )
        in_maps.append(m)

    res = run_bass_kernel_spmd(nc, in_maps, core_ids=list(range(NCORES)))
    yflat = np.concatenate(
        [r["y"].astype(np.float32) for r in res.results], axis=0
    )  # [B, YTOT]
    y = np.empty((B, F, IRREP), dtype=np.float32)
    for l in LORDER:
        d = DS[l]
        seg = yflat[:, YOFF[l] : YOFF[l] + YLEN[l]].reshape(B, d, d, 64)
        y[:, :, OFFS[l] : OFFS[l] + d * d] = seg.transpose(0, 3, 2, 1).reshape(
            B, F, d * d
        )
    return y
